# revision 3
# baseline (speedup 1.0000x reference)
"""Trainium2 Bass kernel for nn_BinaryLinear (binarized linear layer).

Computes: out = sign(x) @ sign(weight - threshold).T * 2^round(clip(shift_param, -8, 0))
with sign(v) = +1 if v >= 0 else -1, for x [32768, 512], weight [512, 512].

Strategy (data-parallel, 8 NeuronCores):
  - Shard x along the token dim: 4096 tokens per core. Replicate weight/threshold.
  - Shards are laid out feature-major (xT = shard.T) so the contraction dim
    (in_features) lands on SBUF partitions without any on-device transpose.
  - On device: binarize x and (weight - threshold) into {-0.5, +0.5} bf16.
    Products are then +-0.25 and PSUM accumulates exact multiples of 0.25
    (|sum| <= 128), so the bf16 matmul is EXACT. The epilogue multiplies by
    4 * 2^round(clip(shift_param)) (a power of two) -> bit-exact f32 result.
  - Matmul: lhsT = xq tile [i128, n128] (stationary), rhs = wq [i128, o512]
    (moving) -> PSUM [n128, o512] in the natural output layout.
"""

from contextlib import ExitStack

import numpy as np

import concourse.bass as bass
import concourse.tile as tile
from concourse import bacc, mybir
from concourse.bass_utils import run_bass_kernel_spmd

N_CORES = 8
TOKENS = 32768
SHARD = TOKENS // N_CORES  # 4096 tokens per core
F_IN = 512
F_OUT = 512
P = 128
KO = F_IN // P  # 4 contraction chunks of 128
NBLK = 512  # tokens per pipeline block
BLOCKS = SHARD // NBLK  # 8
NSUB = NBLK // P  # 4 matmul groups (of 128 tokens) per block

# Results of the last run_bass_kernel_spmd call (for test harnesses to read
# exec_time_ns / profile info when BASS_TRACE=1).
LAST_RESULTS = None
# Extra kwargs test harnesses may inject for run_bass_kernel_spmd
# (e.g. {"trace": True, "tmpdir": ...}). Empty for normal runs.
RUN_KWARGS = {}


def _build_program(scale: float):
    """Build the per-core Bass program. `scale` is baked in as an immediate."""
    nc = bacc.Bacc(
        "TRN2",
        target_bir_lowering=False,
        debug=False,
        num_devices=N_CORES,
    )

    xT = nc.dram_tensor("xT", [F_IN, SHARD], mybir.dt.float32, kind="ExternalInput").ap()
    wT = nc.dram_tensor("wT", [F_IN, F_OUT], mybir.dt.float32, kind="ExternalInput").ap()
    thr = nc.dram_tensor("thr", [P, F_OUT], mybir.dt.float32, kind="ExternalInput").ap()
    out = nc.dram_tensor("out", [SHARD, F_OUT], mybir.dt.float32, kind="ExternalOutput").ap()

    # i = ko*128 + p on partitions
    xT_t = xT.rearrange("(ko p) n -> p ko n", p=P)
    wT_t = wT.rearrange("(ko p) o -> p ko o", p=P)
    # token n = b*NBLK + ns*128 + p
    out_t = out.rearrange("(b ns p) o -> b p ns o", p=P, ns=NSUB)

    with tile.TileContext(nc) as tc:
        with ExitStack() as ctx:
            consts = ctx.enter_context(tc.tile_pool(name="consts", bufs=1))
            xf_pool = ctx.enter_context(tc.tile_pool(name="xf", bufs=3))
            xq_pool = ctx.enter_context(tc.tile_pool(name="xq", bufs=3))
            out_pool = ctx.enter_context(tc.tile_pool(name="outp", bufs=3))
            psum_pool = ctx.enter_context(tc.tile_pool(name="psum", bufs=8, space="PSUM"))

            # --- weights: load f32, binarize to {-0.5, +0.5} bf16 ---
            wf = consts.tile([P, KO, F_OUT], mybir.dt.float32)
            nc.sync.dma_start(wf[:], wT_t)
            th = consts.tile([P, F_OUT], mybir.dt.float32)
            nc.sync.dma_start(th[:], thr)
            wq = consts.tile([P, KO, F_OUT], mybir.dt.bfloat16)
            for k in range(KO):
                # (wT - thr >= 0) -> {0, 1}
                nc.vector.tensor_tensor(
                    wq[:, k], wf[:, k], th[:], mybir.AluOpType.is_ge
                )
            # {0,1} -> {-0.5, +0.5}
            nc.vector.tensor_scalar(
                wq[:], wq[:], -0.5, None, mybir.AluOpType.add
            )

            # --- main pipeline over token blocks ---
            for b in range(BLOCKS):
                xf = xf_pool.tile([P, KO, NBLK], mybir.dt.float32)
                nc.sync.dma_start(xf[:], xT_t[:, :, bass.ts(b, NBLK)])

                # (x >= 0) - 0.5 -> {-0.5, +0.5} in one DVE op
                xq = xq_pool.tile([P, KO, NBLK], mybir.dt.bfloat16)
                nc.vector.tensor_scalar(
                    xq[:], xf[:], 0.0, -0.5,
                    mybir.AluOpType.is_ge, mybir.AluOpType.add,
                )

                ob = out_pool.tile([P, NSUB, F_OUT], mybir.dt.float32)
                for ns in range(NSUB):
                    ps = psum_pool.tile([P, F_OUT], mybir.dt.float32)
                    for k in range(KO):
                        nc.tensor.matmul(
                            ps[:],
                            xq[:, k, bass.ts(ns, P)],
                            wq[:, k],
                            start=(k == 0),
                            stop=(k == KO - 1),
                        )
                    # psum holds sum/4; epilogue applies 4*s (exact power of 2)
                    nc.scalar.mul(ob[:, ns], ps[:], 4.0 * scale)
                nc.sync.dma_start(out_t[b], ob[:])

    nc.compile()
    return nc


def _shift_scale(shift_param) -> float:
    v = np.clip(np.float64(np.asarray(shift_param)), -8.0, 0.0)
    return float(2.0 ** np.round(v))


def make_in_maps(x, weight, threshold):
    x = np.ascontiguousarray(np.asarray(x, dtype=np.float32))
    weight = np.asarray(weight, dtype=np.float32)
    threshold = np.asarray(threshold, dtype=np.float32)

    wT = np.ascontiguousarray(weight.T)  # [in, out]
    thr_b = np.ascontiguousarray(
        np.broadcast_to(threshold.reshape(1, F_OUT), (P, F_OUT))
    ).astype(np.float32)

    in_maps = []
    for c in range(N_CORES):
        shard = x[c * SHARD : (c + 1) * SHARD]  # [SHARD, F_IN]
        xT = np.ascontiguousarray(shard.T)  # [F_IN, SHARD]
        in_maps.append({"xT": xT, "wT": wT, "thr": thr_b})
    return in_maps


def kernel(x, weight, threshold, shift_param) -> np.ndarray:
    global LAST_RESULTS
    scale = _shift_scale(shift_param)
    nc = _build_program(scale)
    in_maps = make_in_maps(x, weight, threshold)
    res = run_bass_kernel_spmd(nc, in_maps, list(range(N_CORES)), **RUN_KWARGS)
    LAST_RESULTS = res
    out = np.concatenate(
        [res.results[c]["out"] for c in range(N_CORES)], axis=0
    )
    return np.ascontiguousarray(out.astype(np.float32, copy=False))
